# revision 26
# baseline (speedup 1.0000x reference)
"""CostVolume2D Trainium2 Bass kernel (v2 — 5-engine pipeline).

cost[b,h,w,d] = sum_c |feat_l[b,h,w,c] - feat_r[b,h,w-d,c]|
(feat_r zero-padded on the left: for w < d the cost is sum_c |feat_l|)

Sharding: pure data-parallel over batch B=8 across 8 NeuronCores (one image
per core); full inputs in, full output out; host does layout prep + gather.

Design (per core), replacing the v1 all-DVE kernel (sim ~952us; this
version sims at ~270us and is verified bit-close on hardware):
  - Transposed layout: partition p = (h%4)*32 + c, so each SBUF column
    holds 4 h-rows x 32 channels. Host stages feat_l/feat_r in this layout
    in bf16 (rel-err budget 2e-2 makes bf16 safe; measured ~3.2e-3 on HW).
  - The C-reduction runs on the TensorEngine: 8 masked ones-weight matmuls
    ([128,32] lhsT, one per hg row of a chunk) accumulate into a PSUM
    quarter-bank, yielding a DENSE [32,512] cost tile (partition = h row).
    Masked weights (ones only in column 4j+h4) let the matmuls pack PSUM
    densely, so evacuation is two cheap [128,512] activation-copies per
    bank (disparities d and d+6 share a bank in opposite halves).
  - Elementwise |l - r| is DVE + ScalarE only (walrus rejects elementwise
    TensorTensor/TensorScalarPtr on GPSIMD, and DMA-CCE accumulate fails
    at runtime, so no third engine is available): DVE runs every subtract
    (bf16 tensor_sub, 2x mode) plus ~1/3 of the abs as uint32 bitwise-AND
    sign-clears (single-src 2x_2p, exact for bf16); ScalarE runs the other
    ~2/3 of the abs as Abs activations plus the PSUM evacuations
    (activation Copy, f32->bf16).
  - The disparity shift is a free-dim offset into a left-zero-padded copy
    of feat_r. DVE 2x mode needs 4-byte-aligned operands, so odd
    disparities read from a second host-staged copy shifted by one element
    (feat_r2), keeping every subtract offset even.
  - Work is 96 fills (8 chunks of 8 hg-rows x 12 disparities); per fill:
    1 sub + 1 abs + 8 matmuls; per chunk-pair and d%6: 1 paired copy +
    2 stores. Chunk inputs are 5-deep, absdiff tiles 8-deep; loads ride
    per-tensor semaphores so waits are minimal.

Engine busy model per core (sim): DVE ~250us (96 subs + 35 abs, 93%
occupancy), ACT ~236us (61 abs + 24 copies), PE ~167us (768 matmuls),
SP/DMA ~100us. Output is staged per-core as [D*H, W] bf16; host casts/
transposes to [B, H, W, D] f32.
"""

from contextlib import ExitStack

import numpy as np
import ml_dtypes

import concourse.bass as bass
import concourse.mybir as mybir
from concourse.bass_utils import run_bass_kernel_spmd

B, H, W, C, D = 8, 256, 512, 32, 12
N_CORES = 8
P = 128
HGT = H // 4          # 64 hg rows (groups of 4 h-rows)
CHG = 8               # hg rows per chunk
NCH = HGT // CHG      # 8 chunks per rep
NFILL = NCH * D       # 96 fills per rep
WPAD = W + D          # 524 padded row length
NBUF = 8              # absdiff ring buffers
NCB = 5               # chunk input buffer depth

BF16 = mybir.dt.bfloat16
F32 = mybir.dt.float32
U32 = mybir.dt.uint32
ABS_MASK = 0x7FFF7FFF



_NC_CACHE = {}


def _schedule(reps):
    """Static per-fill abs-engine assignment ("dve" or "act").

    walrus rejects elementwise TensorTensor/TensorScalarPtr on the Pool
    engine, so only DVE (subs + cheap u32-AND abs) and ScalarE (Abs
    activation + PSUM-evac copies) share the elementwise work. Greedy
    balance with per-fill costs: DVE sub 2.19us, DVE abs 1.10us, ACT abs
    3.6us, ACT copy 0.61us (one per two fills)."""
    NG = NFILL * reps
    abs_eng = []
    dve_load = 0.0
    act_load = 0.0
    for g in range(NG):
        dve_load += 2.19
        if g % 4 == 3:
            act_load += 0.61  # one paired PSUM-evac copy per 4 fills
        if g >= NG - 3 or dve_load + 1.10 <= act_load + 3.6:
            # the tail drains fastest when the last fills' abs rides the
            # DVE queue right behind their subtracts
            abs_eng.append("dve")
            dve_load += 1.10
        else:
            abs_eng.append("act")
            act_load += 3.6
    return abs_eng


def build_nc(reps=1):
    nc = bass.Bass(detect_race_conditions=False)
    fl = nc.dram_tensor("feat_l_t", [P, HGT * W], BF16, kind="ExternalInput")
    fr = nc.dram_tensor("feat_r_t", [P, HGT * WPAD], BF16, kind="ExternalInput")
    fr2 = nc.dram_tensor("feat_r2_t", [P, HGT * WPAD], BF16, kind="ExternalInput")
    wts = nc.dram_tensor("wts", [P, CHG * 32], BF16, kind="ExternalInput")
    cost = nc.dram_tensor("cost_t", [D * H, W], BF16, kind="ExternalOutput")

    NG = NFILL * reps
    NCG = NCH * reps           # total chunk loads
    NK = 6 * (NCH // 2) * reps   # total copies/stores: (pair q, d%6)
    abs_eng = _schedule(reps)

    # per-engine cumulative ranks (all subs run on DVE in fill order, so
    # sub_sem counts fills directly; dve-abs and act-abs each count their
    # own completions on a dedicated semaphore)
    act_fills = [g for g in range(NG) if abs_eng[g] == "act"]
    act_rank = {g: i for i, g in enumerate(act_fills)}
    dveab_fills = [g for g in range(NG) if abs_eng[g] == "dve"]
    dveab_rank = {g: i for i, g in enumerate(dveab_fills)}

    # copies: k = 6q + dd (chunk pair q = c8//2, dd = d % 6): one [128, 512]
    # activation-copy evacuates bank dd (rows 0:64 = disparity dd, rows
    # 64:128 = disparity dd+6) once fills (2q+1, dd) and (2q+1, dd+6) are
    # both done, i.e. after fill 24q+18+dd. Deadline for the ACT merge:
    act_events = sorted(
        [("abs", g, g) for g in act_fills]
        + [("copy", k, 24 * (k // 6) + 20 + (k % 6)) for k in range(NK)],
        key=lambda e: (e[2], 0 if e[0] == "abs" else 1, e[1]),
    )

    # SP-ring loads per chunk in order [L, R, (W,) R2], each tensor on its
    # own semaphore. A fill of chunk cg needs l_sem>=16(cg+1) and r_sem
    # (even d) / r2_sem (odd d) >=16(cg+1).

    psum = [nc.alloc_psum_tensor(f"ps{i}", [P, W], F32) for i in range(6)]

    stack = ExitStack()
    Lb = [
        stack.enter_context(nc.sbuf_tensor(f"Lb{i}", [P, CHG * W], BF16))
        for i in range(NCB)
    ]
    Rb = [
        stack.enter_context(nc.sbuf_tensor(f"Rb{i}", [P, CHG * WPAD], BF16))
        for i in range(NCB)
    ]
    R2b = [
        stack.enter_context(nc.sbuf_tensor(f"R2b{i}", [P, CHG * WPAD], BF16))
        for i in range(NCB)
    ]
    AD = [
        stack.enter_context(nc.sbuf_tensor(f"AD{i}", [P, CHG * W], BF16))
        for i in range(NBUF)
    ]

    def r_slice(g, bi):
        d = g % D
        if d % 2 == 0:
            src, off = Rb, D - d
        else:
            src, off = R2b, D - 1 - d
        R3 = src[bi][:, :].rearrange("p (hg w) -> p hg w", hg=CHG)
        return R3[:, :, off : off + W]

    def quarter(g):
        """PSUM target of fill g: (bank, base_partition)."""
        c8 = g // D
        d = g % D
        return d % 6, 64 * (d // 6) + 32 * (c8 % 2)

    with (
        stack,
        nc.sbuf_tensor([P, CHG * 32], BF16) as Ws,
        nc.sbuf_tensor([P, W], BF16) as OS0,
        nc.sbuf_tensor([P, W], BF16) as OS1,
        nc.semaphore("l_sem") as l_sem,
        nc.semaphore("r_sem") as r_sem,
        nc.semaphore("r2_sem") as r2_sem,
        nc.semaphore("w_sem") as w_sem,
        nc.semaphore("sub_sem") as sub_sem,
        nc.semaphore("act_sem") as act_sem,
        nc.semaphore("dveab_sem") as dveab_sem,
        nc.semaphore("mm_sem") as mm_sem,
        nc.semaphore("cp_sem") as cp_sem,
        nc.semaphore("st_sem") as st_sem,
        nc.Block() as block,
    ):
        OSb = [OS0, OS1]

        @block.sync
        def _(sync):
            def load_chunk(cg):
                c = cg % NCH
                bi = cg % NCB
                if cg >= NCB:
                    # WAR: fills of chunk cg-NCB fully consumed by PE
                    sync.wait_ge(mm_sem, D * (cg - NCB + 1))
                sync.dma_start(
                    out=Lb[bi][:, :], in_=fl[:, c * CHG * W : (c + 1) * CHG * W]
                ).then_inc(l_sem, 16)
                sync.dma_start(
                    out=Rb[bi][:, :], in_=fr[:, c * CHG * WPAD : (c + 1) * CHG * WPAD]
                ).then_inc(r_sem, 16)
                if cg == 0:
                    sync.dma_start(out=Ws[:, :], in_=wts[:, :]).then_inc(w_sem, 16)
                sync.dma_start(
                    out=R2b[bi][:, :],
                    in_=fr2[:, c * CHG * WPAD : (c + 1) * CHG * WPAD],
                ).then_inc(r2_sem, 16)

            for cg in range(min(NCB, NCG)):
                load_chunk(cg)
            next_cg = NCB
            for k in range(NK):
                q, dd = k // 6, k % 6
                # approx fill time of store k, for prefetch pacing
                fk = 24 * q + 18 + dd
                while next_cg < NCG and fk >= 12 * (next_cg - NCB + 1) + 2:
                    load_chunk(next_cg)
                    next_cg += 1
                sync.wait_ge(cp_sem, k + 1)
                qh = (q % 4) * 64
                sync.dma_start(
                    out=cost[dd * H + qh : dd * H + qh + 64, :],
                    in_=OSb[k % 2][0:64, :],
                ).then_inc(st_sem, 16)
                sync.dma_start(
                    out=cost[(dd + 6) * H + qh : (dd + 6) * H + qh + 64, :],
                    in_=OSb[k % 2][64:128, :],
                ).then_inc(st_sem, 16)
            sync.wait_ge(st_sem, 32 * NK)
            for s in (
                l_sem,
                r_sem,
                r2_sem,
                w_sem,
                sub_sem,
                act_sem,
                dveab_sem,
                mm_sem,
                cp_sem,
                st_sem,
            ):
                sync.sem_clear(s)

        @block.vector
        def _(vector):
            for g in range(NG):
                cg = g // D
                bi = cg % NCB
                ab = g % NBUF
                vector.wait_ge(l_sem, 16 * (cg + 1))
                if (g % D) % 2 == 0:
                    vector.wait_ge(r_sem, 16 * (cg + 1))
                else:
                    vector.wait_ge(r2_sem, 16 * (cg + 1))
                if g >= NBUF:
                    vector.wait_ge(mm_sem, g - NBUF + 1)
                vector.tensor_sub(
                    AD[ab][:, :], Lb[bi][:, :], r_slice(g, bi)
                ).then_inc(sub_sem, 1)
                if abs_eng[g] == "dve":
                    adu = AD[ab][:, :].bitcast(U32)
                    vector.tensor_scalar(
                        adu, adu, ABS_MASK, None, mybir.AluOpType.bitwise_and
                    ).then_inc(dveab_sem, 1)

        @block.scalar
        def _(scalar):
            for kind, i, _dl in act_events:
                if kind == "abs":
                    g = i
                    scalar.wait_ge(sub_sem, g + 1)
                    ab = g % NBUF
                    scalar.activation(
                        AD[ab][:, :], AD[ab][:, :], mybir.ActivationFunctionType.Abs
                    ).then_inc(act_sem, 1)
                else:
                    k = i
                    q, dd = k // 6, k % 6
                    # all four quarters of bank dd for pair q are done
                    scalar.wait_ge(mm_sem, 24 * q + 18 + dd + 1)
                    if k >= 2:
                        scalar.wait_ge(st_sem, 32 * (k - 1))
                    scalar.activation(
                        OSb[k % 2][:, :],
                        psum[dd][:, :],
                        mybir.ActivationFunctionType.Copy,
                    ).then_inc(cp_sem, 1)

        # copy k is fireable once fill 24*(k//6)+18+(k%6) completed; a fill
        # 5+ past that point touches the same PSUM bank the copy reads,
        # which the interp flags bank-wide. Throttle PE to stay within 5
        # fills of the fireable-copy frontier (also covers the bank reuse
        # from pair q-1).
        fire_fill = sorted(24 * (k // 6) + 19 + (k % 6) for k in range(NK))

        def cp_need(g):
            import bisect

            n = bisect.bisect_right(fire_fill, g - 5)
            q, d = g // (2 * D), g % D
            if q >= 1:
                n = max(n, 6 * (q - 1) + (d % 6) + 1)
            return n

        @block.tensor
        def _(tensor):
            tensor.wait_ge(w_sem, 16)
            W3 = Ws[:, :].rearrange("p (j m) -> p j m", j=CHG)
            for g in range(NG):
                d = g % D
                ab = g % NBUF
                if abs_eng[g] == "act":
                    tensor.wait_ge(act_sem, act_rank[g] + 1)
                else:
                    tensor.wait_ge(dveab_sem, dveab_rank[g] + 1)
                if cp_need(g) > 0:
                    tensor.wait_ge(cp_sem, cp_need(g))
                bank, base = quarter(g)
                AD3 = AD[ab][:, :].rearrange("p (j w) -> p j w", j=CHG)
                out_ap = psum[bank][base : base + 32, :]
                for j in range(CHG):
                    inst = tensor.matmul(
                        out=out_ap,
                        lhsT=W3[:, j, :],
                        rhs=AD3[:, j, :],
                        start=(j == 0),
                        stop=(j == CHG - 1),
                        tile_position=(0, base),
                    )
                inst.then_inc(mm_sem, 1)

    return nc


def _get_nc():
    if "nc" not in _NC_CACHE:
        _NC_CACHE["nc"] = build_nc()
    return _NC_CACHE["nc"]


def _stage_inputs(feat_l, feat_r):
    """Host-side layout prep. Returns per-core input maps."""
    feat_l = np.asarray(feat_l, dtype=np.float32)
    feat_r = np.asarray(feat_r, dtype=np.float32)
    # masked ones-weights: W[k, j, 4j + k//32] = 1  (j = hg row in chunk)
    wt = np.zeros((P, CHG, 32), dtype=ml_dtypes.bfloat16)
    k = np.arange(P)
    for j in range(CHG):
        wt[k, j, 4 * j + k // 32] = 1.0
    wt = np.ascontiguousarray(wt.reshape(P, CHG * 32))

    in_maps = []
    for b in range(B):
        # [H, W, C] -> (hg, h4, w, c) -> (h4, c, hg, w) -> [128, HGT*W]
        lt = (
            feat_l[b]
            .reshape(HGT, 4, W, C)
            .transpose(1, 3, 0, 2)
            .astype(ml_dtypes.bfloat16)
        )
        rt = (
            feat_r[b]
            .reshape(HGT, 4, W, C)
            .transpose(1, 3, 0, 2)
            .astype(ml_dtypes.bfloat16)
        )
        rp = np.zeros((4, C, HGT, WPAD), dtype=ml_dtypes.bfloat16)
        rp[:, :, :, D:] = rt
        rp2 = np.zeros((4, C, HGT, WPAD), dtype=ml_dtypes.bfloat16)
        rp2[:, :, :, D - 1 : D - 1 + W] = rt
        in_maps.append(
            {
                "feat_l_t": np.ascontiguousarray(lt.reshape(P, HGT * W)),
                "feat_r_t": np.ascontiguousarray(rp.reshape(P, HGT * WPAD)),
                "feat_r2_t": np.ascontiguousarray(rp2.reshape(P, HGT * WPAD)),
                "wts": wt,
            }
        )
    return in_maps


def _gather_output(results):
    out = np.empty((B, H, W, D), dtype=np.float32)
    for b in range(B):
        ct = np.asarray(results[b]["cost_t"]).astype(np.float32)
        out[b] = ct.reshape(D, H, W).transpose(1, 2, 0)
    return out


def _run(feat_l, feat_r, trace=False, nc=None):
    if nc is None:
        nc = _get_nc()
    in_maps = _stage_inputs(feat_l, feat_r)
    res = run_bass_kernel_spmd(nc, in_maps, list(range(N_CORES)), trace=trace)
    return _gather_output(res.results), res


def kernel(feat_l, feat_r):
    out, _ = _run(feat_l, feat_r, trace=False)
    return out


# revision 32
# speedup vs baseline: 1.0125x; 1.0125x over previous
"""CostVolume2D Trainium2 Bass kernel (v2 — 5-engine pipeline).

cost[b,h,w,d] = sum_c |feat_l[b,h,w,c] - feat_r[b,h,w-d,c]|
(feat_r zero-padded on the left: for w < d the cost is sum_c |feat_l|)

Sharding: pure data-parallel over batch B=8 across 8 NeuronCores (one image
per core); full inputs in, full output out; host does layout prep + gather.

Design (per core), replacing the v1 all-DVE kernel (sim ~952us; this
version sims at ~270us and is verified bit-close on hardware):
  - Transposed layout: partition p = (h%4)*32 + c, so each SBUF column
    holds 4 h-rows x 32 channels. Host stages feat_l/feat_r in this layout
    in bf16 (rel-err budget 2e-2 makes bf16 safe; measured ~3.2e-3 on HW).
  - The C-reduction runs on the TensorEngine: 8 masked ones-weight matmuls
    ([128,32] lhsT, one per hg row of a chunk) accumulate into a PSUM
    quarter-bank, yielding a DENSE [32,512] cost tile (partition = h row).
    Masked weights (ones only in column 4j+h4) let the matmuls pack PSUM
    densely, so evacuation is two cheap [128,512] activation-copies per
    bank (disparities d and d+6 share a bank in opposite halves).
  - Elementwise |l - r| is DVE + ScalarE only (walrus rejects elementwise
    TensorTensor/TensorScalarPtr on GPSIMD, and DMA-CCE accumulate fails
    at runtime, so no third engine is available): DVE runs every subtract
    (bf16 tensor_sub, 2x mode) plus ~1/3 of the abs as uint32 bitwise-AND
    sign-clears (single-src 2x_2p, exact for bf16); ScalarE runs the other
    ~2/3 of the abs as Abs activations plus the PSUM evacuations
    (activation Copy, f32->bf16).
  - The disparity shift is a free-dim offset into a left-zero-padded copy
    of feat_r. DVE 2x mode needs 4-byte-aligned operands, so odd
    disparities read from a second host-staged copy shifted by one element
    (feat_r2), keeping every subtract offset even.
  - Work is 96 fills (8 chunks of 8 hg-rows x 12 disparities); per fill:
    1 sub + 1 abs + 8 matmuls; per chunk-pair and d%6: 1 paired copy +
    2 stores. Chunk inputs are 5-deep, absdiff tiles 8-deep; loads ride
    per-tensor semaphores so waits are minimal.

Engine busy model per core (sim): DVE ~250us (96 subs + 35 abs, 93%
occupancy), ACT ~236us (61 abs + 24 copies), PE ~167us (768 matmuls),
SP/DMA ~100us. Output is staged per-core as [D*H, W] bf16; host casts/
transposes to [B, H, W, D] f32.
"""

from contextlib import ExitStack

import numpy as np
import ml_dtypes

import concourse.bass as bass
import concourse.mybir as mybir
from concourse.bass_utils import run_bass_kernel_spmd

B, H, W, C, D = 8, 256, 512, 32, 12
N_CORES = 8
P = 128
HGT = H // 4          # 64 hg rows (groups of 4 h-rows)
CHG = 8               # hg rows per chunk
NCH = HGT // CHG      # 8 chunks per rep
NFILL = NCH * D       # 96 fills per rep
WPAD = W + D          # 524 padded row length
NBUF = 8              # absdiff ring buffers
NCB = 5               # chunk input buffer depth

BF16 = mybir.dt.bfloat16
F32 = mybir.dt.float32
U32 = mybir.dt.uint32
ABS_MASK = 0x7FFF7FFF



_NC_CACHE = {}


def _schedule(reps):
    """Static per-fill abs-engine assignment ("dve" or "act").

    walrus rejects elementwise TensorTensor/TensorScalarPtr on the Pool
    engine, so only DVE (subs + cheap u32-AND abs) and ScalarE (Abs
    activation + PSUM-evac copies) share the elementwise work. Greedy
    balance with per-fill costs: DVE sub 2.19us, DVE abs 1.10us, ACT abs
    3.6us, ACT copy 0.61us (one per two fills)."""
    NG = NFILL * reps
    abs_eng = []
    dve_load = 0.0
    act_load = 0.0
    for g in range(NG):
        dve_load += 2.19
        if g % 4 == 3:
            act_load += 0.61  # one paired PSUM-evac copy per 4 fills
        if g >= NG - 3 or dve_load + 1.10 <= act_load + 3.6:
            # the tail drains fastest when the last fills' abs rides the
            # DVE queue right behind their subtracts
            abs_eng.append("dve")
            dve_load += 1.10
        else:
            abs_eng.append("act")
            act_load += 3.6
    return abs_eng


def build_nc(reps=1):
    nc = bass.Bass(detect_race_conditions=False)
    fl = nc.dram_tensor("feat_l_t", [P, HGT * W], BF16, kind="ExternalInput")
    fr = nc.dram_tensor("feat_r_t", [P, HGT * WPAD], BF16, kind="ExternalInput")
    fr2 = nc.dram_tensor("feat_r2_t", [P, HGT * WPAD], BF16, kind="ExternalInput")
    wts = nc.dram_tensor("wts", [P, CHG * 32], BF16, kind="ExternalInput")
    cost = nc.dram_tensor("cost_t", [D * H, W], BF16, kind="ExternalOutput")

    NG = NFILL * reps
    NCG = NCH * reps           # total chunk loads
    NK = 6 * (NCH // 2) * reps   # total copies/stores: (pair q, d%6)
    abs_eng = _schedule(reps)

    # per-engine cumulative ranks (all subs run on DVE in fill order, so
    # sub_sem counts fills directly; dve-abs and act-abs each count their
    # own completions on a dedicated semaphore)
    act_fills = [g for g in range(NG) if abs_eng[g] == "act"]
    act_rank = {g: i for i, g in enumerate(act_fills)}
    dveab_fills = [g for g in range(NG) if abs_eng[g] == "dve"]
    dveab_rank = {g: i for i, g in enumerate(dveab_fills)}

    # copies: k = 6q + dd (chunk pair q = c8//2, dd = d % 6): one [128, 512]
    # activation-copy evacuates bank dd (rows 0:64 = disparity dd, rows
    # 64:128 = disparity dd+6) once fills (2q+1, dd) and (2q+1, dd+6) are
    # both done, i.e. after fill 24q+18+dd. Deadline for the ACT merge:
    act_events = sorted(
        [("abs", g, g) for g in act_fills]
        + [("copy", k, 24 * (k // 6) + 20 + (k % 6)) for k in range(NK)],
        key=lambda e: (e[2], 0 if e[0] == "abs" else 1, e[1]),
    )

    # SP-ring loads per chunk in order [L, R, (W,) R2], each tensor on its
    # own semaphore. A fill of chunk cg needs l_sem>=16(cg+1) and r_sem
    # (even d) / r2_sem (odd d) >=16(cg+1).

    psum = [nc.alloc_psum_tensor(f"ps{i}", [P, W], F32) for i in range(6)]

    stack = ExitStack()
    Lb = [
        stack.enter_context(nc.sbuf_tensor(f"Lb{i}", [P, CHG * W], BF16))
        for i in range(NCB)
    ]
    Rb = [
        stack.enter_context(nc.sbuf_tensor(f"Rb{i}", [P, CHG * WPAD], BF16))
        for i in range(NCB)
    ]
    R2b = [
        stack.enter_context(nc.sbuf_tensor(f"R2b{i}", [P, CHG * WPAD], BF16))
        for i in range(NCB)
    ]
    AD = [
        stack.enter_context(nc.sbuf_tensor(f"AD{i}", [P, CHG * W], BF16))
        for i in range(NBUF)
    ]

    def r_slice(g, bi):
        d = g % D
        if d % 2 == 0:
            src, off = Rb, D - d
        else:
            src, off = R2b, D - 1 - d
        R3 = src[bi][:, :].rearrange("p (hg w) -> p hg w", hg=CHG)
        return R3[:, :, off : off + W]

    def quarter(g):
        """PSUM target of fill g: (bank, base_partition)."""
        c8 = g // D
        d = g % D
        return d % 6, 64 * (d // 6) + 32 * (c8 % 2)

    with (
        stack,
        nc.sbuf_tensor([P, CHG * 32], BF16) as Ws,
        nc.sbuf_tensor([P, W], BF16) as OS0,
        nc.sbuf_tensor([P, W], BF16) as OS1,
        nc.semaphore("l_sem") as l_sem,
        nc.semaphore("r_sem") as r_sem,
        nc.semaphore("r2_sem") as r2_sem,
        nc.semaphore("w_sem") as w_sem,
        nc.semaphore("sub_sem") as sub_sem,
        nc.semaphore("act_sem") as act_sem,
        nc.semaphore("dveab_sem") as dveab_sem,
        nc.semaphore("mm_sem") as mm_sem,
        nc.semaphore("cp_sem") as cp_sem,
        nc.semaphore("st_sem") as st_sem,
        nc.Block() as block,
    ):
        OSb = [OS0, OS1]

        @block.sync
        def _(sync):
            def load_chunk(cg):
                c = cg % NCH
                bi = cg % NCB
                if cg >= NCB:
                    # WAR: fills of chunk cg-NCB fully consumed by PE
                    sync.wait_ge(mm_sem, D * (cg - NCB + 1))
                sync.dma_start(
                    out=Lb[bi][:, :], in_=fl[:, c * CHG * W : (c + 1) * CHG * W]
                ).then_inc(l_sem, 16)
                if cg > 0:
                    sync.dma_start(
                        out=Rb[bi][:, :],
                        in_=fr[:, c * CHG * WPAD : (c + 1) * CHG * WPAD],
                    ).then_inc(r_sem, 16)
                sync.dma_start(
                    out=R2b[bi][:, :],
                    in_=fr2[:, c * CHG * WPAD : (c + 1) * CHG * WPAD],
                ).then_inc(r2_sem, 16)

            for cg in range(min(NCB, NCG)):
                load_chunk(cg)
            next_cg = NCB
            for k in range(NK):
                q, dd = k // 6, k % 6
                # approx fill time of store k, for prefetch pacing
                fk = 24 * q + 18 + dd
                while next_cg < NCG and fk >= 12 * (next_cg - NCB + 1) + 2:
                    load_chunk(next_cg)
                    next_cg += 1
                sync.wait_ge(cp_sem, k + 1)
                qh = (q % 4) * 64
                sync.dma_start(
                    out=cost[dd * H + qh : dd * H + qh + 64, :],
                    in_=OSb[k % 2][0:64, :],
                ).then_inc(st_sem, 16)
                sync.dma_start(
                    out=cost[(dd + 6) * H + qh : (dd + 6) * H + qh + 64, :],
                    in_=OSb[k % 2][64:128, :],
                ).then_inc(st_sem, 16)
            sync.wait_ge(st_sem, 32 * NK)
            for s in (
                l_sem,
                r_sem,
                r2_sem,
                w_sem,
                sub_sem,
                act_sem,
                dveab_sem,
                mm_sem,
                cp_sem,
                st_sem,
            ):
                sync.sem_clear(s)

        @block.vector
        def _(vector):
            for g in range(NG):
                cg = g // D
                bi = cg % NCB
                ab = g % NBUF
                vector.wait_ge(l_sem, 16 * (cg + 1))
                if (g % D) % 2 == 0:
                    vector.wait_ge(r_sem, 16 * (cg + 1))
                else:
                    vector.wait_ge(r2_sem, 16 * (cg + 1))
                if g >= NBUF:
                    vector.wait_ge(mm_sem, g - NBUF + 1)
                vector.tensor_sub(
                    AD[ab][:, :], Lb[bi][:, :], r_slice(g, bi)
                ).then_inc(sub_sem, 1)
                if abs_eng[g] == "dve":
                    adu = AD[ab][:, :].bitcast(U32)
                    vector.tensor_scalar(
                        adu, adu, ABS_MASK, None, mybir.AluOpType.bitwise_and
                    ).then_inc(dveab_sem, 1)

        @block.scalar
        def _(scalar):
            # bootstrap: chunk-0 R + weights on the ACT HWDGE ring, parallel
            # with SP's L0 load (ACT is otherwise idle until the first abs)
            scalar.dma_start(out=Rb[0][:, :], in_=fr[:, : CHG * WPAD]).then_inc(
                r_sem, 16
            )
            scalar.dma_start(out=Ws[:, :], in_=wts[:, :]).then_inc(w_sem, 16)
            for kind, i, _dl in act_events:
                if kind == "abs":
                    g = i
                    scalar.wait_ge(sub_sem, g + 1)
                    ab = g % NBUF
                    scalar.activation(
                        AD[ab][:, :], AD[ab][:, :], mybir.ActivationFunctionType.Abs
                    ).then_inc(act_sem, 1)
                else:
                    k = i
                    q, dd = k // 6, k % 6
                    # all four quarters of bank dd for pair q are done
                    scalar.wait_ge(mm_sem, 24 * q + 18 + dd + 1)
                    if k >= 2:
                        scalar.wait_ge(st_sem, 32 * (k - 1))
                    scalar.activation(
                        OSb[k % 2][:, :],
                        psum[dd][:, :],
                        mybir.ActivationFunctionType.Copy,
                    ).then_inc(cp_sem, 1)

        # copy k is fireable once fill 24*(k//6)+18+(k%6) completed; a fill
        # 5+ past that point touches the same PSUM bank the copy reads,
        # which the interp flags bank-wide. Throttle PE to stay within 5
        # fills of the fireable-copy frontier (also covers the bank reuse
        # from pair q-1).
        fire_fill = sorted(24 * (k // 6) + 19 + (k % 6) for k in range(NK))

        def cp_need(g):
            import bisect

            n = bisect.bisect_right(fire_fill, g - 5)
            q, d = g // (2 * D), g % D
            if q >= 1:
                n = max(n, 6 * (q - 1) + (d % 6) + 1)
            return n

        @block.tensor
        def _(tensor):
            tensor.wait_ge(w_sem, 16)
            W3 = Ws[:, :].rearrange("p (j m) -> p j m", j=CHG)
            for g in range(NG):
                d = g % D
                ab = g % NBUF
                if abs_eng[g] == "act":
                    tensor.wait_ge(act_sem, act_rank[g] + 1)
                else:
                    tensor.wait_ge(dveab_sem, dveab_rank[g] + 1)
                if cp_need(g) > 0:
                    tensor.wait_ge(cp_sem, cp_need(g))
                bank, base = quarter(g)
                AD3 = AD[ab][:, :].rearrange("p (j w) -> p j w", j=CHG)
                out_ap = psum[bank][base : base + 32, :]
                for j in range(CHG):
                    inst = tensor.matmul(
                        out=out_ap,
                        lhsT=W3[:, j, :],
                        rhs=AD3[:, j, :],
                        start=(j == 0),
                        stop=(j == CHG - 1),
                        tile_position=(0, base),
                    )
                inst.then_inc(mm_sem, 1)

    return nc


def _get_nc():
    if "nc" not in _NC_CACHE:
        _NC_CACHE["nc"] = build_nc()
    return _NC_CACHE["nc"]


def _stage_inputs(feat_l, feat_r):
    """Host-side layout prep. Returns per-core input maps."""
    feat_l = np.asarray(feat_l, dtype=np.float32)
    feat_r = np.asarray(feat_r, dtype=np.float32)
    # masked ones-weights: W[k, j, 4j + k//32] = 1  (j = hg row in chunk)
    wt = np.zeros((P, CHG, 32), dtype=ml_dtypes.bfloat16)
    k = np.arange(P)
    for j in range(CHG):
        wt[k, j, 4 * j + k // 32] = 1.0
    wt = np.ascontiguousarray(wt.reshape(P, CHG * 32))

    in_maps = []
    for b in range(B):
        # [H, W, C] -> (hg, h4, w, c) -> (h4, c, hg, w) -> [128, HGT*W]
        lt = (
            feat_l[b]
            .reshape(HGT, 4, W, C)
            .transpose(1, 3, 0, 2)
            .astype(ml_dtypes.bfloat16)
        )
        rt = (
            feat_r[b]
            .reshape(HGT, 4, W, C)
            .transpose(1, 3, 0, 2)
            .astype(ml_dtypes.bfloat16)
        )
        rp = np.zeros((4, C, HGT, WPAD), dtype=ml_dtypes.bfloat16)
        rp[:, :, :, D:] = rt
        rp2 = np.zeros((4, C, HGT, WPAD), dtype=ml_dtypes.bfloat16)
        rp2[:, :, :, D - 1 : D - 1 + W] = rt
        in_maps.append(
            {
                "feat_l_t": np.ascontiguousarray(lt.reshape(P, HGT * W)),
                "feat_r_t": np.ascontiguousarray(rp.reshape(P, HGT * WPAD)),
                "feat_r2_t": np.ascontiguousarray(rp2.reshape(P, HGT * WPAD)),
                "wts": wt,
            }
        )
    return in_maps


def _gather_output(results):
    out = np.empty((B, H, W, D), dtype=np.float32)
    for b in range(B):
        ct = np.asarray(results[b]["cost_t"]).astype(np.float32)
        out[b] = ct.reshape(D, H, W).transpose(1, 2, 0)
    return out


def _run(feat_l, feat_r, trace=False, nc=None):
    if nc is None:
        nc = _get_nc()
    in_maps = _stage_inputs(feat_l, feat_r)
    res = run_bass_kernel_spmd(nc, in_maps, list(range(N_CORES)), trace=trace)
    return _gather_output(res.results), res


def kernel(feat_l, feat_r):
    out, _ = _run(feat_l, feat_r, trace=False)
    return out
